# revision 35
# baseline (speedup 1.0000x reference)
"""Trainium2 Bass kernel for nn_CnUpdateLayer (LDPC check-node update).

Math: out[b,i] = prod_{j: mask[i,j]!=0} x[b,j], or 0 if mask row i is empty.
Since mask is exactly {0,1} and x ~ randn (no exact zeros), the masked product
is computed in log-domain via matmul:

    L[b,i]  = sum_j ln|x[b,j]| * mask[i,j]       (magnitude, log domain)
    C[b,i]  = sum_j [x[b,j]<0] * mask[i,j]       (negative count)
    deg[i]  = sum_j mask[i,j]                    (row degree)
    out     = exp(.5*Lhi)*exp(.5*Llo) * (min(deg,1) - 2*(C&1))

ln(x^2) is split hi/lo into two bf16 halves so the matmul runs at bf16 rate
while keeping ~fp32 accuracy.  The stationary operand is
[ln_hi | ln_lo | signbits | ones*32] = 128 columns: the 32 identical "ones"
columns replicate the row degree into PSUM partitions 96:128, so the epilogue
needs no K=1 broadcast matmul.

Schedule (final):
  - The whole 1MB mask streams on the SWDGE (gpsimd) ring in 4 groups in
    matmul consumption order: under load that ring sustains 160-230 GB/s
    while each HWDGE ring manages only ~40-80, and one continuous stream
    avoids per-ring contention.  x (fp16) + sign bits ride the sync HWDGE
    ring; the scalar ring stays free for ACT work until the output.  W is
    stored column-major [128, 128cols, KC] so the sign-bit DMA and ones
    memset write contiguous bytes per partition (a strided 64B-element dest
    shatters the DMA into tiny packets and chokes every ring) while each
    chunk's lhsT stays a single-stride AP.  Each transfer's semaphore fires
    ~1.3-2us after its last byte (completion receipt) - the matmul phase is
    paced by group receipts.
  - No PE warm-up: the hot 2.4GHz p-state is unreachable (HW DVFS needs
    sustained real utilization, so matmuls run at 1.2GHz), and a dummy
    bridge FIFO-delays the real matmuls for more than the ~180ns one
    cold-state matmul costs.
  - ln-prep: 4 pipelined blocks; DVE does squares + hi-casts + lo, ACT only
    the 4 Ln's.  GpSimd ALU is never used (~30x slower than DVE and its
    SBUF traffic interferes with DVE).
  - Epilogue (~2.4us, DVE-queue-bound): four PSUM readers in order
    lo, Lhi+lo, 2C->int32, min(deg,1)-(2C&2), then ONE exp(.5*(Lhi+Llo)).
    Mixed PSUM+SBUF operands at equal partition base are legal (the
    equal-base rule only binds when BOTH inputs are SBUF).  Output leaves
    in two halves on the two HWDGE rings.
"""

import sys

if "/opt/trn_rl_repo" not in sys.path:
    sys.path.insert(0, "/opt/trn_rl_repo")

import numpy as np

B = 32          # batch codewords
IN_F = 2048     # input edges
OUT_F = 2048    # output edges
NCORES = 8
SHARD = OUT_F // NCORES     # 256 output edges per core
KC = IN_F // 128            # 16 contraction chunks of 128
PB = 4                      # prep block size (chunks)
WHI, WLO, WSGN, WONE = 0, B, 2 * B, 3 * B       # 0, 32, 64, 96
WTOT = 4 * B                                    # 128

_PROG = None


def _build_program():
    import concourse.tile as tile
    from concourse import bacc, mybir
    from concourse.alu_op_type import AluOpType

    F32 = mybir.dt.float32
    F16 = mybir.dt.float16
    I32 = mybir.dt.int32
    BF16 = mybir.dt.bfloat16
    AF = mybir.ActivationFunctionType

    nc = bacc.Bacc("TRN2", target_bir_lowering=False)
    xt = nc.dram_tensor("xt", [128, KC * B], F16, kind="ExternalInput")
    st = nc.dram_tensor("st", [128, KC * B], BF16, kind="ExternalInput")
    mt = nc.dram_tensor("mt", [128, KC * SHARD], BF16, kind="ExternalInput")
    out = nc.dram_tensor("out", [B, SHARD], F32, kind="ExternalOutput")

    with tile.TileContext(nc) as tc:
        with (
            tc.tile_pool(name="pool", bufs=1) as pool,
            tc.tile_pool(name="psum", bufs=1, space="PSUM") as psum_pool,
        ):
            # ---- input DMAs.  x + sign bits on the scalar HWDGE ring (ahead
            # of the Ln table load, which overlaps their transfer); the mask
            # alone on the sync ring in 4 groups of 4 chunks so arrival order
            # matches matmul consumption order.
            # dummy Ln first on the scalar queue: its 1.28us ACT_TABLE_LOAD
            # runs while x streams on the sync ring, and naturally delays the
            # scalar-ring mask issues so x gets the full DMA bandwidth first.
            dmy = pool.tile([1, 1], F32)
            nc.vector.memset(dmy, 1.0)
            dln = pool.tile([1, 1], F32)
            nc.scalar.activation(out=dln, in_=dmy, func=AF.Ln)

            x_sb = pool.tile([128, B, KC], F16)
            nc.sync.dma_start(
                out=x_sb, in_=xt.ap().rearrange("p (b c) -> p b c", c=KC))
            w_sb = pool.tile([128, WTOT, KC], BF16)
            nc.sync.dma_start(
                out=w_sb[:, WSGN:WSGN + B, :],
                in_=st.ap().rearrange("p (b c) -> p b c", c=KC))
            m_sb = pool.tile([128, KC, SHARD], BF16)
            mt_v = mt.ap().rearrange("p (c n) -> p c n", n=SHARD)
            nc.gpsimd.dma_start(out=m_sb[:, 0:4, :], in_=mt_v[:, 0:4, :])
            nc.gpsimd.dma_start(out=m_sb[:, 4:8, :], in_=mt_v[:, 4:8, :])
            nc.gpsimd.dma_start(out=m_sb[:, 8:12, :], in_=mt_v[:, 8:12, :])
            nc.gpsimd.dma_start(out=m_sb[:, 12:16, :], in_=mt_v[:, 12:16, :])

            # the replicated-ones block of W: DVE memset, no deps,
            # scheduled right after the preamble.
            nc.vector.memset(w_sb[:, WONE:WONE + B, :], 1.0)

            # ---- stationary operand W = [hi | lo | sgn | ones], bf16, in 4
            # pipelined blocks of 4 chunks.  ln|x| = ln(x^2) (x^2 on DVE
            # avoids the Abs table); the 0.5 is folded into the Exp scale.
            # ACT does only the Ln's; DVE everything else.
            sq_sb = pool.tile([128, B, KC], F32)
            ln_sb = pool.tile([128, B, KC], F32)
            for h in range(0, KC, PB):
                sl = slice(h, h + PB)
                nc.vector.tensor_tensor(
                    out=sq_sb[:, :, sl], in0=x_sb[:, :, sl], in1=x_sb[:, :, sl],
                    op=AluOpType.mult)
                nc.scalar.activation(out=ln_sb[:, :, sl], in_=sq_sb[:, :, sl], func=AF.Ln)
                nc.vector.tensor_scalar(
                    out=w_sb[:, WHI:WHI + B, sl], in0=ln_sb[:, :, sl],
                    scalar1=0.0, scalar2=None, op0=AluOpType.add)
                nc.vector.tensor_tensor(
                    out=w_sb[:, WLO:WLO + B, sl], in0=ln_sb[:, :, sl],
                    in1=w_sb[:, WHI:WHI + B, sl], op=AluOpType.subtract)

            # dummy Exp AFTER the Ln phase (input reads ln_sb to pin the
            # ordering): its table load overlaps the matmuls instead of
            # stalling the real Exps.
            dex = pool.tile([1, 1], F32)
            nc.scalar.activation(out=dex, in_=ln_sb[0:1, 0:1, KC - 1], func=AF.Exp)

            # ---- main accumulation: ps[0:128] += W_c^T @ M_c over 16 chunks.
            # Rows 0:32 = Lhi, 32:64 = Llo, 64:96 = C, 96:128 = deg (x32).
            ps = psum_pool.tile([128, SHARD], F32)
            for c in range(KC):
                nc.tensor.matmul(
                    ps, lhsT=w_sb[:, :, c], rhs=m_sb[:, c, :],
                    start=(c == 0), stop=(c == KC - 1))

            # ---- epilogue.  PSUM readers of one bank serialize, so keep
            # them few and short, and put the Lhi+Llo sum BEFORE a single Exp:
            #   ci2  = 2*C as int32                     (PSUM reader 1)
            #   lo   = Llo drained to SBUF              (PSUM reader 2)
            #   ll   = Lhi + lo (one PSUM + one SBUF operand at equal base -
            #          mixed-space inputs skip the equal-base rule) (reader 3)
            #   zv   = min(deg,1) - (ci2 & 2)           (PSUM reader 4)
            #   out  = exp(.5*ll) * zv                  (ONE Exp, not two)
            # deg==0 implies C==0, so empty rows get exactly 0.
            lo_sb = pool.tile([B, SHARD], F32)
            nc.vector.tensor_scalar(
                out=lo_sb, in0=ps[WLO:WLO + B, :],
                scalar1=0.0, scalar2=None, op0=AluOpType.add)
            ll = pool.tile([B, SHARD], F32)
            nc.vector.tensor_tensor(
                out=ll, in0=ps[WHI:WHI + B, :], in1=lo_sb, op=AluOpType.add)
            ci2 = pool.tile([B, SHARD], I32)
            nc.vector.tensor_scalar(
                out=ci2, in0=ps[WSGN:WSGN + B, :],
                scalar1=2.0, scalar2=None, op0=AluOpType.mult)
            odd2 = pool.tile([B, SHARD], I32)
            nc.vector.tensor_scalar(
                out=odd2, in0=ci2, scalar1=2, scalar2=None,
                op0=AluOpType.bitwise_and)
            a = pool.tile([B, SHARD], F32)
            nc.scalar.activation(out=a, in_=ll, func=AF.Exp, scale=0.5)
            zv = pool.tile([B, SHARD], F32)
            nc.vector.scalar_tensor_tensor(
                out=zv, in0=ps[WONE:WONE + B, :], scalar=1.0, in1=odd2,
                op0=AluOpType.min, op1=AluOpType.subtract)
            o_sb = pool.tile([B, SHARD], F32)
            nc.vector.tensor_tensor(out=o_sb, in0=a, in1=zv, op=AluOpType.mult)
            # fp32 output (fp16 underflows: products reach ~1e-9, below the
            # fp16 subnormal floor); two halves on the two HWDGE rings
            H = SHARD // 2
            nc.sync.dma_start(out=out.ap()[:, 0:H], in_=o_sb[:, 0:H])
            nc.scalar.dma_start(out=out.ap()[:, H:SHARD], in_=o_sb[:, H:SHARD])

    nc.compile()
    return nc


def _get_program():
    global _PROG
    if _PROG is None:
        _PROG = _build_program()
    return _PROG


def _prep_inputs(x, mask):
    import ml_dtypes

    x = np.ascontiguousarray(x, dtype=np.float32)
    mask = np.ascontiguousarray(mask, dtype=np.float32)
    # xt[p, b*KC + c] = x[b, c*128 + p]
    xtf = np.ascontiguousarray(
        x.T.reshape(KC, 128, B).transpose(1, 2, 0).reshape(128, B * KC))
    st = (xtf < 0).astype(ml_dtypes.bfloat16)
    xt = xtf.astype(np.float16)
    mask_bf = mask.astype(ml_dtypes.bfloat16)
    in_maps = []
    for k in range(NCORES):
        shard = mask_bf[k * SHARD:(k + 1) * SHARD, :]      # [256, 2048]
        # mt[p, c*SHARD + n] = mask[k*SHARD + n, c*128 + p]
        mt = np.ascontiguousarray(
            shard.T.reshape(KC, 128, SHARD).transpose(1, 0, 2).reshape(128, KC * SHARD))
        in_maps.append({"xt": xt, "st": st, "mt": mt})
    return in_maps


def run(x, mask, trace=False):
    """Run on 8 NeuronCores; returns (output, BassKernelResults)."""
    from concourse.bass_utils import run_bass_kernel_spmd

    nc = _get_program()
    in_maps = _prep_inputs(x, mask)
    res = run_bass_kernel_spmd(nc, in_maps, core_ids=list(range(NCORES)), trace=trace)
    out = np.concatenate([np.asarray(r["out"], dtype=np.float32)
                          for r in res.results], axis=1)
    return np.ascontiguousarray(out, dtype=np.float32), res


def kernel(x, mask):
    out, _ = run(x, mask, trace=False)
    return out


# revision 36
# speedup vs baseline: 1.0897x; 1.0897x over previous
"""Trainium2 Bass kernel for nn_CnUpdateLayer (LDPC check-node update).

Math: out[b,i] = prod_{j: mask[i,j]!=0} x[b,j], or 0 if mask row i is empty.
Since mask is exactly {0,1} and x ~ randn (no exact zeros), the masked product
is computed in log-domain via matmul:

    L[b,i]  = sum_j ln|x[b,j]| * mask[i,j]       (magnitude, log domain)
    C[b,i]  = sum_j [x[b,j]<0] * mask[i,j]       (negative count)
    deg[i]  = sum_j mask[i,j]                    (row degree)
    out     = exp(.5*Lhi)*exp(.5*Llo) * (min(deg,1) - 2*(C&1))

ln(x^2) is split hi/lo into two bf16 halves so the matmul runs at bf16 rate
while keeping ~fp32 accuracy.  The stationary operand is
[ln_hi | ln_lo | signbits | ones*32] = 128 columns: the 32 identical "ones"
columns replicate the row degree into PSUM partitions 96:128, so the epilogue
needs no K=1 broadcast matmul.

Schedule (final):
  - The whole 1MB mask streams on the SWDGE (gpsimd) ring in 4 groups in
    matmul consumption order: under load that ring sustains 160-230 GB/s
    while each HWDGE ring manages only ~40-80, and one continuous stream
    avoids per-ring contention.  x (fp16) + sign bits ride the sync HWDGE
    ring; the scalar ring stays free for ACT work until the output.  W is
    stored column-major [128, 128cols, KC] so the sign-bit DMA and ones
    memset write contiguous bytes per partition (a strided 64B-element dest
    shatters the DMA into tiny packets and chokes every ring) while each
    chunk's lhsT stays a single-stride AP.  Each transfer's semaphore fires
    ~1.3-2us after its last byte (completion receipt) - the matmul phase is
    paced by group receipts.
  - No PE warm-up: the hot 2.4GHz p-state is unreachable (HW DVFS needs
    sustained real utilization, so matmuls run at 1.2GHz), and a dummy
    bridge FIFO-delays the real matmuls for more than the ~180ns one
    cold-state matmul costs.
  - ln-prep: 4 pipelined blocks; DVE does squares + hi-casts + lo, ACT only
    the 4 Ln's.  GpSimd ALU is never used (~30x slower than DVE and its
    SBUF traffic interferes with DVE).
  - Epilogue (~2.4us, DVE-queue-bound): four PSUM readers in order
    lo, Lhi+lo, 2C->int32, min(deg,1)-(2C&2), then ONE exp(.5*(Lhi+Llo)).
    Mixed PSUM+SBUF operands at equal partition base are legal (the
    equal-base rule only binds when BOTH inputs are SBUF).  Output leaves
    in two halves on the two HWDGE rings.
"""

import sys

if "/opt/trn_rl_repo" not in sys.path:
    sys.path.insert(0, "/opt/trn_rl_repo")

import numpy as np

B = 32          # batch codewords
IN_F = 2048     # input edges
OUT_F = 2048    # output edges
NCORES = 8
SHARD = OUT_F // NCORES     # 256 output edges per core
KC = IN_F // 128            # 16 contraction chunks of 128
PB = 4                      # prep block size (chunks)
WHI, WLO, WSGN, WONE = 0, B, 2 * B, 3 * B       # 0, 32, 64, 96
WTOT = 4 * B                                    # 128

_PROG = None


def _build_program():
    import concourse.tile as tile
    from concourse import bacc, mybir
    from concourse.alu_op_type import AluOpType

    F32 = mybir.dt.float32
    F16 = mybir.dt.float16
    I32 = mybir.dt.int32
    BF16 = mybir.dt.bfloat16
    AF = mybir.ActivationFunctionType

    nc = bacc.Bacc("TRN2", target_bir_lowering=False)
    xt = nc.dram_tensor("xt", [128, KC * B], F16, kind="ExternalInput")
    st = nc.dram_tensor("st", [128, KC * B], BF16, kind="ExternalInput")
    mt = nc.dram_tensor("mt", [128, KC * SHARD], BF16, kind="ExternalInput")
    out = nc.dram_tensor("out", [B, SHARD], F32, kind="ExternalOutput")

    with tile.TileContext(nc) as tc:
        with (
            tc.tile_pool(name="pool", bufs=1) as pool,
            tc.tile_pool(name="psum", bufs=1, space="PSUM") as psum_pool,
        ):
            # ---- input DMAs.  x + sign bits on the scalar HWDGE ring (ahead
            # of the Ln table load, which overlaps their transfer); the mask
            # alone on the sync ring in 4 groups of 4 chunks so arrival order
            # matches matmul consumption order.
            # dummy Ln first on the scalar queue: its 1.28us ACT_TABLE_LOAD
            # runs while x streams on the sync ring, and naturally delays the
            # scalar-ring mask issues so x gets the full DMA bandwidth first.
            dmy = pool.tile([1, 1], F32)
            nc.vector.memset(dmy, 1.0)
            dln = pool.tile([1, 1], F32)
            nc.scalar.activation(out=dln, in_=dmy, func=AF.Ln)

            x_sb = pool.tile([128, B, KC], F16)
            nc.sync.dma_start(
                out=x_sb, in_=xt.ap().rearrange("p (b c) -> p b c", c=KC))
            w_sb = pool.tile([128, WTOT, KC], BF16)
            nc.sync.dma_start(
                out=w_sb[:, WSGN:WSGN + B, :],
                in_=st.ap().rearrange("p (b c) -> p b c", c=KC))
            m_sb = pool.tile([128, KC, SHARD], BF16)
            mt_v = mt.ap().rearrange("p (c n) -> p c n", n=SHARD)
            nc.gpsimd.dma_start(out=m_sb[:, 0:4, :], in_=mt_v[:, 0:4, :])
            nc.gpsimd.dma_start(out=m_sb[:, 4:8, :], in_=mt_v[:, 4:8, :])
            nc.gpsimd.dma_start(out=m_sb[:, 8:12, :], in_=mt_v[:, 8:12, :])
            nc.gpsimd.dma_start(out=m_sb[:, 12:14, :], in_=mt_v[:, 12:14, :])
            nc.sync.dma_start(out=m_sb[:, 14:16, :], in_=mt_v[:, 14:16, :])

            # the replicated-ones block of W: DVE memset, no deps,
            # scheduled right after the preamble.
            nc.vector.memset(w_sb[:, WONE:WONE + B, :], 1.0)

            # ---- stationary operand W = [hi | lo | sgn | ones], bf16, in 4
            # pipelined blocks of 4 chunks.  ln|x| = ln(x^2) (x^2 on DVE
            # avoids the Abs table); the 0.5 is folded into the Exp scale.
            # ACT does only the Ln's; DVE everything else.
            sq_sb = pool.tile([128, B, KC], F32)
            ln_sb = pool.tile([128, B, KC], F32)
            for h in range(0, KC, PB):
                sl = slice(h, h + PB)
                nc.vector.tensor_tensor(
                    out=sq_sb[:, :, sl], in0=x_sb[:, :, sl], in1=x_sb[:, :, sl],
                    op=AluOpType.mult)
                nc.scalar.activation(out=ln_sb[:, :, sl], in_=sq_sb[:, :, sl], func=AF.Ln)
                nc.vector.tensor_scalar(
                    out=w_sb[:, WHI:WHI + B, sl], in0=ln_sb[:, :, sl],
                    scalar1=0.0, scalar2=None, op0=AluOpType.add)
                nc.vector.tensor_tensor(
                    out=w_sb[:, WLO:WLO + B, sl], in0=ln_sb[:, :, sl],
                    in1=w_sb[:, WHI:WHI + B, sl], op=AluOpType.subtract)

            # dummy Exp AFTER the Ln phase (input reads ln_sb to pin the
            # ordering): its table load overlaps the matmuls instead of
            # stalling the real Exps.
            dex = pool.tile([1, 1], F32)
            nc.scalar.activation(out=dex, in_=ln_sb[0:1, 0:1, KC - 1], func=AF.Exp)

            # ---- main accumulation: ps[0:128] += W_c^T @ M_c over 16 chunks.
            # Rows 0:32 = Lhi, 32:64 = Llo, 64:96 = C, 96:128 = deg (x32).
            ps = psum_pool.tile([128, SHARD], F32)
            for c in range(KC):
                nc.tensor.matmul(
                    ps, lhsT=w_sb[:, :, c], rhs=m_sb[:, c, :],
                    start=(c == 0), stop=(c == KC - 1))

            # ---- epilogue.  PSUM readers of one bank serialize, so keep
            # them few and short, and put the Lhi+Llo sum BEFORE a single Exp:
            #   ci2  = 2*C as int32                     (PSUM reader 1)
            #   lo   = Llo drained to SBUF              (PSUM reader 2)
            #   ll   = Lhi + lo (one PSUM + one SBUF operand at equal base -
            #          mixed-space inputs skip the equal-base rule) (reader 3)
            #   zv   = min(deg,1) - (ci2 & 2)           (PSUM reader 4)
            #   out  = exp(.5*ll) * zv                  (ONE Exp, not two)
            # deg==0 implies C==0, so empty rows get exactly 0.
            lo_sb = pool.tile([B, SHARD], F32)
            nc.vector.tensor_scalar(
                out=lo_sb, in0=ps[WLO:WLO + B, :],
                scalar1=0.0, scalar2=None, op0=AluOpType.add)
            ll = pool.tile([B, SHARD], F32)
            nc.vector.tensor_tensor(
                out=ll, in0=ps[WHI:WHI + B, :], in1=lo_sb, op=AluOpType.add)
            ci2 = pool.tile([B, SHARD], I32)
            nc.vector.tensor_scalar(
                out=ci2, in0=ps[WSGN:WSGN + B, :],
                scalar1=2.0, scalar2=None, op0=AluOpType.mult)
            odd2 = pool.tile([B, SHARD], I32)
            nc.vector.tensor_scalar(
                out=odd2, in0=ci2, scalar1=2, scalar2=None,
                op0=AluOpType.bitwise_and)
            a = pool.tile([B, SHARD], F32)
            nc.scalar.activation(out=a, in_=ll, func=AF.Exp, scale=0.5)
            zv = pool.tile([B, SHARD], F32)
            nc.vector.scalar_tensor_tensor(
                out=zv, in0=ps[WONE:WONE + B, :], scalar=1.0, in1=odd2,
                op0=AluOpType.min, op1=AluOpType.subtract)
            o_sb = pool.tile([B, SHARD], F32)
            nc.vector.tensor_tensor(out=o_sb, in0=a, in1=zv, op=AluOpType.mult)
            # fp32 output (fp16 underflows: products reach ~1e-9, below the
            # fp16 subnormal floor); two halves on the two HWDGE rings
            H = SHARD // 2
            nc.sync.dma_start(out=out.ap()[:, 0:H], in_=o_sb[:, 0:H])
            nc.scalar.dma_start(out=out.ap()[:, H:SHARD], in_=o_sb[:, H:SHARD])

    nc.compile()
    return nc


def _get_program():
    global _PROG
    if _PROG is None:
        _PROG = _build_program()
    return _PROG


def _prep_inputs(x, mask):
    import ml_dtypes

    x = np.ascontiguousarray(x, dtype=np.float32)
    mask = np.ascontiguousarray(mask, dtype=np.float32)
    # xt[p, b*KC + c] = x[b, c*128 + p]
    xtf = np.ascontiguousarray(
        x.T.reshape(KC, 128, B).transpose(1, 2, 0).reshape(128, B * KC))
    st = (xtf < 0).astype(ml_dtypes.bfloat16)
    xt = xtf.astype(np.float16)
    mask_bf = mask.astype(ml_dtypes.bfloat16)
    in_maps = []
    for k in range(NCORES):
        shard = mask_bf[k * SHARD:(k + 1) * SHARD, :]      # [256, 2048]
        # mt[p, c*SHARD + n] = mask[k*SHARD + n, c*128 + p]
        mt = np.ascontiguousarray(
            shard.T.reshape(KC, 128, SHARD).transpose(1, 0, 2).reshape(128, KC * SHARD))
        in_maps.append({"xt": xt, "st": st, "mt": mt})
    return in_maps


def run(x, mask, trace=False):
    """Run on 8 NeuronCores; returns (output, BassKernelResults)."""
    from concourse.bass_utils import run_bass_kernel_spmd

    nc = _get_program()
    in_maps = _prep_inputs(x, mask)
    res = run_bass_kernel_spmd(nc, in_maps, core_ids=list(range(NCORES)), trace=trace)
    out = np.concatenate([np.asarray(r["out"], dtype=np.float32)
                          for r in res.results], axis=1)
    return np.ascontiguousarray(out, dtype=np.float32), res


def kernel(x, mask):
    out, _ = run(x, mask, trace=False)
    return out


# revision 38
# speedup vs baseline: 1.1032x; 1.0124x over previous
"""Trainium2 Bass kernel for nn_CnUpdateLayer (LDPC check-node update).

Math: out[b,i] = prod_{j: mask[i,j]!=0} x[b,j], or 0 if mask row i is empty.
Since mask is exactly {0,1} and x ~ randn (no exact zeros), the masked product
is computed in log-domain via matmul:

    L[b,i]  = sum_j ln|x[b,j]| * mask[i,j]       (magnitude, log domain)
    C[b,i]  = sum_j [x[b,j]<0] * mask[i,j]       (negative count)
    deg[i]  = sum_j mask[i,j]                    (row degree)
    out     = exp(.5*Lhi)*exp(.5*Llo) * (min(deg,1) - 2*(C&1))

ln(x^2) is split hi/lo into two bf16 halves so the matmul runs at bf16 rate
while keeping ~fp32 accuracy.  The stationary operand is
[ln_hi | ln_lo | signbits | ones*32] = 128 columns: the 32 identical "ones"
columns replicate the row degree into PSUM partitions 96:128, so the epilogue
needs no K=1 broadcast matmul.

Schedule (final):
  - The mask streams on the SWDGE (gpsimd) ring in consumption order as
    groups {4,4,4,2}; the final 2-chunk group rides the sync HWDGE ring,
    arriving long before it is needed, so the gpsimd stream (and with it
    the last completion receipt that paces the matmul tail) ends sooner.
    Under load the SWDGE ring sustains 160-230 GB/s while each HWDGE ring
    manages only ~40-80.  x (fp16) + sign bits also ride the sync ring;
    the scalar ring stays free for ACT work until the output.  W is
    stored column-major [128, 128cols, KC] so the sign-bit DMA and ones
    memset write contiguous bytes per partition (a strided 64B-element dest
    shatters the DMA into tiny packets and chokes every ring) while each
    chunk's lhsT stays a single-stride AP.  Each transfer's semaphore fires
    ~1.3-2us after its last byte (completion receipt) - the matmul phase is
    paced by group receipts.
  - No PE warm-up: the hot 2.4GHz p-state is unreachable (HW DVFS needs
    sustained real utilization, so matmuls run at 1.2GHz), and a dummy
    bridge FIFO-delays the real matmuls for more than the ~180ns one
    cold-state matmul costs.
  - ln-prep: 4 pipelined blocks; DVE does squares + hi-casts + lo, ACT only
    the 4 Ln's.  GpSimd ALU is never used (~30x slower than DVE and its
    SBUF traffic interferes with DVE).
  - Epilogue (~2.4us, DVE-queue-bound): four PSUM readers in order
    lo, Lhi+lo, 2C->int32, min(deg,1)-(2C&2), then ONE exp(.5*(Lhi+Llo)).
    Mixed PSUM+SBUF operands at equal partition base are legal (the
    equal-base rule only binds when BOTH inputs are SBUF).  Output leaves
    in two halves on the two HWDGE rings.
"""

import sys

if "/opt/trn_rl_repo" not in sys.path:
    sys.path.insert(0, "/opt/trn_rl_repo")

import numpy as np

B = 32          # batch codewords
IN_F = 2048     # input edges
OUT_F = 2048    # output edges
NCORES = 8
SHARD = OUT_F // NCORES     # 256 output edges per core
KC = IN_F // 128            # 16 contraction chunks of 128
PB = 4                      # prep block size (chunks)
WHI, WLO, WSGN, WONE = 0, B, 2 * B, 3 * B       # 0, 32, 64, 96
WTOT = 4 * B                                    # 128

_PROG = None


def _build_program():
    import concourse.tile as tile
    from concourse import bacc, mybir
    from concourse.alu_op_type import AluOpType

    F32 = mybir.dt.float32
    F16 = mybir.dt.float16
    I32 = mybir.dt.int32
    BF16 = mybir.dt.bfloat16
    AF = mybir.ActivationFunctionType

    nc = bacc.Bacc("TRN2", target_bir_lowering=False)
    xt = nc.dram_tensor("xt", [128, KC * B], F16, kind="ExternalInput")
    st = nc.dram_tensor("st", [128, KC * B], BF16, kind="ExternalInput")
    mt = nc.dram_tensor("mt", [128, KC * SHARD], BF16, kind="ExternalInput")
    out = nc.dram_tensor("out", [B, SHARD], F32, kind="ExternalOutput")

    with tile.TileContext(nc) as tc:
        with (
            tc.tile_pool(name="pool", bufs=1) as pool,
            tc.tile_pool(name="psum", bufs=1, space="PSUM") as psum_pool,
        ):
            # ---- input DMAs.  x + sign bits on the scalar HWDGE ring (ahead
            # of the Ln table load, which overlaps their transfer); the mask
            # alone on the sync ring in 4 groups of 4 chunks so arrival order
            # matches matmul consumption order.
            # dummy Ln first on the scalar queue: its 1.28us ACT_TABLE_LOAD
            # runs while x streams on the sync ring, and naturally delays the
            # scalar-ring mask issues so x gets the full DMA bandwidth first.
            dmy = pool.tile([1, 1], F32)
            nc.vector.memset(dmy, 1.0)
            dln = pool.tile([1, 1], F32)
            nc.scalar.activation(out=dln, in_=dmy, func=AF.Ln)

            x_sb = pool.tile([128, B, KC], F16)
            nc.sync.dma_start(
                out=x_sb, in_=xt.ap().rearrange("p (b c) -> p b c", c=KC))
            w_sb = pool.tile([128, WTOT, KC], BF16)
            nc.sync.dma_start(
                out=w_sb[:, WSGN:WSGN + B, :],
                in_=st.ap().rearrange("p (b c) -> p b c", c=KC))
            m_sb = pool.tile([128, KC, SHARD], BF16)
            mt_v = mt.ap().rearrange("p (c n) -> p c n", n=SHARD)
            nc.gpsimd.dma_start(out=m_sb[:, 0:4, :], in_=mt_v[:, 0:4, :])
            nc.gpsimd.dma_start(out=m_sb[:, 4:6, :], in_=mt_v[:, 4:6, :])
            nc.gpsimd.dma_start(out=m_sb[:, 6:8, :], in_=mt_v[:, 6:8, :])
            nc.gpsimd.dma_start(out=m_sb[:, 8:10, :], in_=mt_v[:, 8:10, :])
            nc.gpsimd.dma_start(out=m_sb[:, 10:12, :], in_=mt_v[:, 10:12, :])
            nc.gpsimd.dma_start(out=m_sb[:, 12:14, :], in_=mt_v[:, 12:14, :])
            nc.sync.dma_start(out=m_sb[:, 14:16, :], in_=mt_v[:, 14:16, :])

            # the replicated-ones block of W: DVE memset, no deps,
            # scheduled right after the preamble.
            nc.vector.memset(w_sb[:, WONE:WONE + B, :], 1.0)

            # ---- stationary operand W = [hi | lo | sgn | ones], bf16, in 4
            # pipelined blocks of 4 chunks.  ln|x| = ln(x^2) (x^2 on DVE
            # avoids the Abs table); the 0.5 is folded into the Exp scale.
            # ACT does only the Ln's; DVE everything else.
            sq_sb = pool.tile([128, B, KC], F32)
            ln_sb = pool.tile([128, B, KC], F32)
            for h in range(0, KC, PB):
                sl = slice(h, h + PB)
                nc.vector.tensor_tensor(
                    out=sq_sb[:, :, sl], in0=x_sb[:, :, sl], in1=x_sb[:, :, sl],
                    op=AluOpType.mult)
                nc.scalar.activation(out=ln_sb[:, :, sl], in_=sq_sb[:, :, sl], func=AF.Ln)
                nc.vector.tensor_scalar(
                    out=w_sb[:, WHI:WHI + B, sl], in0=ln_sb[:, :, sl],
                    scalar1=0.0, scalar2=None, op0=AluOpType.add)
                nc.vector.tensor_tensor(
                    out=w_sb[:, WLO:WLO + B, sl], in0=ln_sb[:, :, sl],
                    in1=w_sb[:, WHI:WHI + B, sl], op=AluOpType.subtract)

            # dummy Exp AFTER the Ln phase (input reads ln_sb to pin the
            # ordering): its table load overlaps the matmuls instead of
            # stalling the real Exps.
            dex = pool.tile([1, 1], F32)
            nc.scalar.activation(out=dex, in_=ln_sb[0:1, 0:1, KC - 1], func=AF.Exp)

            # ---- main accumulation: ps[0:128] += W_c^T @ M_c over 16 chunks.
            # Rows 0:32 = Lhi, 32:64 = Llo, 64:96 = C, 96:128 = deg (x32).
            ps = psum_pool.tile([128, SHARD], F32)
            for c in range(KC):
                nc.tensor.matmul(
                    ps, lhsT=w_sb[:, :, c], rhs=m_sb[:, c, :],
                    start=(c == 0), stop=(c == KC - 1))

            # ---- epilogue.  PSUM readers of one bank serialize, so keep
            # them few and short, and put the Lhi+Llo sum BEFORE a single Exp:
            #   ci2  = 2*C as int32                     (PSUM reader 1)
            #   lo   = Llo drained to SBUF              (PSUM reader 2)
            #   ll   = Lhi + lo (one PSUM + one SBUF operand at equal base -
            #          mixed-space inputs skip the equal-base rule) (reader 3)
            #   zv   = min(deg,1) - (ci2 & 2)           (PSUM reader 4)
            #   out  = exp(.5*ll) * zv                  (ONE Exp, not two)
            # deg==0 implies C==0, so empty rows get exactly 0.
            lo_sb = pool.tile([B, SHARD], F32)
            nc.vector.tensor_scalar(
                out=lo_sb, in0=ps[WLO:WLO + B, :],
                scalar1=0.0, scalar2=None, op0=AluOpType.add)
            ll = pool.tile([B, SHARD], F32)
            nc.vector.tensor_tensor(
                out=ll, in0=ps[WHI:WHI + B, :], in1=lo_sb, op=AluOpType.add)
            ci2 = pool.tile([B, SHARD], I32)
            nc.vector.tensor_scalar(
                out=ci2, in0=ps[WSGN:WSGN + B, :],
                scalar1=2.0, scalar2=None, op0=AluOpType.mult)
            odd2 = pool.tile([B, SHARD], I32)
            nc.vector.tensor_scalar(
                out=odd2, in0=ci2, scalar1=2, scalar2=None,
                op0=AluOpType.bitwise_and)
            a = pool.tile([B, SHARD], F32)
            nc.scalar.activation(out=a, in_=ll, func=AF.Exp, scale=0.5)
            zv = pool.tile([B, SHARD], F32)
            nc.vector.scalar_tensor_tensor(
                out=zv, in0=ps[WONE:WONE + B, :], scalar=1.0, in1=odd2,
                op0=AluOpType.min, op1=AluOpType.subtract)
            o_sb = pool.tile([B, SHARD], F32)
            nc.vector.tensor_tensor(out=o_sb, in0=a, in1=zv, op=AluOpType.mult)
            # fp32 output (fp16 underflows: products reach ~1e-9, below the
            # fp16 subnormal floor); two halves on the two HWDGE rings
            H = SHARD // 2
            nc.sync.dma_start(out=out.ap()[:, 0:H], in_=o_sb[:, 0:H])
            nc.scalar.dma_start(out=out.ap()[:, H:SHARD], in_=o_sb[:, H:SHARD])

    nc.compile()
    return nc


def _get_program():
    global _PROG
    if _PROG is None:
        _PROG = _build_program()
    return _PROG


def _prep_inputs(x, mask):
    import ml_dtypes

    x = np.ascontiguousarray(x, dtype=np.float32)
    mask = np.ascontiguousarray(mask, dtype=np.float32)
    # xt[p, b*KC + c] = x[b, c*128 + p]
    xtf = np.ascontiguousarray(
        x.T.reshape(KC, 128, B).transpose(1, 2, 0).reshape(128, B * KC))
    st = (xtf < 0).astype(ml_dtypes.bfloat16)
    xt = xtf.astype(np.float16)
    mask_bf = mask.astype(ml_dtypes.bfloat16)
    in_maps = []
    for k in range(NCORES):
        shard = mask_bf[k * SHARD:(k + 1) * SHARD, :]      # [256, 2048]
        # mt[p, c*SHARD + n] = mask[k*SHARD + n, c*128 + p]
        mt = np.ascontiguousarray(
            shard.T.reshape(KC, 128, SHARD).transpose(1, 0, 2).reshape(128, KC * SHARD))
        in_maps.append({"xt": xt, "st": st, "mt": mt})
    return in_maps


def run(x, mask, trace=False):
    """Run on 8 NeuronCores; returns (output, BassKernelResults)."""
    from concourse.bass_utils import run_bass_kernel_spmd

    nc = _get_program()
    in_maps = _prep_inputs(x, mask)
    res = run_bass_kernel_spmd(nc, in_maps, core_ids=list(range(NCORES)), trace=trace)
    out = np.concatenate([np.asarray(r["out"], dtype=np.float32)
                          for r in res.results], axis=1)
    return np.ascontiguousarray(out, dtype=np.float32), res


def kernel(x, mask):
    out, _ = run(x, mask, trace=False)
    return out


# revision 39
# speedup vs baseline: 1.1753x; 1.0653x over previous
"""Trainium2 Bass kernel for nn_CnUpdateLayer (LDPC check-node update).

Math: out[b,i] = prod_{j: mask[i,j]!=0} x[b,j], or 0 if mask row i is empty.
Since mask is exactly {0,1} and x ~ randn (no exact zeros), the masked product
is computed in log-domain via matmul:

    L[b,i]  = sum_j ln|x[b,j]| * mask[i,j]       (magnitude, log domain)
    C[b,i]  = sum_j [x[b,j]<0] * mask[i,j]       (negative count)
    deg[i]  = sum_j mask[i,j]                    (row degree)
    out     = exp(.5*Lhi)*exp(.5*Llo) * (min(deg,1) - 2*(C&1))

ln(x^2) is split hi/lo into two bf16 halves so the matmul runs at bf16 rate
while keeping ~fp32 accuracy.  The stationary operand is
[ln_hi | ln_lo | signbits | ones*32] = 128 columns: the 32 identical "ones"
columns replicate the row degree into PSUM partitions 96:128, so the epilogue
needs no K=1 broadcast matmul.

Schedule (final):
  - The mask streams on the SWDGE (gpsimd) ring in consumption order as
    groups {4,4,4,2}; the final 2-chunk group rides the sync HWDGE ring,
    arriving long before it is needed, so the gpsimd stream (and with it
    the last completion receipt that paces the matmul tail) ends sooner.
    Under load the SWDGE ring sustains 160-230 GB/s while each HWDGE ring
    manages only ~40-80.  x (fp16) + sign bits also ride the sync ring;
    the scalar ring stays free for ACT work until the output.  W is
    stored column-major [128, 128cols, KC] so the sign-bit DMA and ones
    memset write contiguous bytes per partition (a strided 64B-element dest
    shatters the DMA into tiny packets and chokes every ring) while each
    chunk's lhsT stays a single-stride AP.  Each transfer's semaphore fires
    ~1.3-2us after its last byte (completion receipt) - the matmul phase is
    paced by group receipts.
  - No PE warm-up: the hot 2.4GHz p-state is unreachable (HW DVFS needs
    sustained real utilization, so matmuls run at 1.2GHz), and a dummy
    bridge FIFO-delays the real matmuls for more than the ~180ns one
    cold-state matmul costs.
  - ln-prep: 4 pipelined blocks; DVE does squares + hi-casts + lo, ACT only
    the 4 Ln's.  GpSimd ALU is never used (~30x slower than DVE and its
    SBUF traffic interferes with DVE).
  - Epilogue (~2.4us, DVE-queue-bound): four PSUM readers in order
    lo, Lhi+lo, 2C->int32, min(deg,1)-(2C&2), then ONE exp(.5*(Lhi+Llo)).
    Mixed PSUM+SBUF operands at equal partition base are legal (the
    equal-base rule only binds when BOTH inputs are SBUF).  Output leaves
    in two halves on the two HWDGE rings.
"""

import sys

if "/opt/trn_rl_repo" not in sys.path:
    sys.path.insert(0, "/opt/trn_rl_repo")

import numpy as np

B = 32          # batch codewords
IN_F = 2048     # input edges
OUT_F = 2048    # output edges
NCORES = 8
SHARD = OUT_F // NCORES     # 256 output edges per core
KC = IN_F // 128            # 16 contraction chunks of 128
PB = 4                      # prep block size (chunks)
WHI, WLO, WSGN, WONE = 0, B, 2 * B, 3 * B       # 0, 32, 64, 96
WTOT = 4 * B                                    # 128

_PROG = None


def _build_program():
    import concourse.tile as tile
    from concourse import bacc, mybir
    from concourse.alu_op_type import AluOpType

    F32 = mybir.dt.float32
    F16 = mybir.dt.float16
    I32 = mybir.dt.int32
    BF16 = mybir.dt.bfloat16
    AF = mybir.ActivationFunctionType

    nc = bacc.Bacc("TRN2", target_bir_lowering=False)
    xt = nc.dram_tensor("xt", [128, KC * B], F16, kind="ExternalInput")
    st = nc.dram_tensor("st", [128, KC * B], BF16, kind="ExternalInput")
    mt = nc.dram_tensor("mt", [128, KC * SHARD], BF16, kind="ExternalInput")
    out = nc.dram_tensor("out", [B, SHARD], F32, kind="ExternalOutput")

    with tile.TileContext(nc) as tc:
        with (
            tc.tile_pool(name="pool", bufs=1) as pool,
            tc.tile_pool(name="psum", bufs=1, space="PSUM") as psum_pool,
        ):
            # ---- input DMAs.  x + sign bits on the scalar HWDGE ring (ahead
            # of the Ln table load, which overlaps their transfer); the mask
            # alone on the sync ring in 4 groups of 4 chunks so arrival order
            # matches matmul consumption order.
            # dummy Ln first on the scalar queue: its 1.28us ACT_TABLE_LOAD
            # runs while x streams on the sync ring, and naturally delays the
            # scalar-ring mask issues so x gets the full DMA bandwidth first.
            dmy = pool.tile([1, 1], F32)
            nc.vector.memset(dmy, 1.0)
            dln = pool.tile([1, 1], F32)
            nc.scalar.activation(out=dln, in_=dmy, func=AF.Ln)

            x_sb = pool.tile([128, B, KC], F16)
            nc.sync.dma_start(
                out=x_sb, in_=xt.ap().rearrange("p (b c) -> p b c", c=KC))
            w_sb = pool.tile([128, WTOT, KC], BF16)
            nc.sync.dma_start(
                out=w_sb[:, WSGN:WSGN + B, :],
                in_=st.ap().rearrange("p (b c) -> p b c", c=KC))
            m_sb = pool.tile([128, KC, SHARD], BF16)
            mt_v = mt.ap().rearrange("p (c n) -> p c n", n=SHARD)
            nc.gpsimd.dma_start(out=m_sb[:, 0:2, :], in_=mt_v[:, 0:2, :])
            nc.gpsimd.dma_start(out=m_sb[:, 2:4, :], in_=mt_v[:, 2:4, :])
            nc.gpsimd.dma_start(out=m_sb[:, 4:6, :], in_=mt_v[:, 4:6, :])
            nc.gpsimd.dma_start(out=m_sb[:, 6:8, :], in_=mt_v[:, 6:8, :])
            nc.gpsimd.dma_start(out=m_sb[:, 8:10, :], in_=mt_v[:, 8:10, :])
            nc.gpsimd.dma_start(out=m_sb[:, 10:12, :], in_=mt_v[:, 10:12, :])
            nc.gpsimd.dma_start(out=m_sb[:, 12:14, :], in_=mt_v[:, 12:14, :])
            nc.sync.dma_start(out=m_sb[:, 14:16, :], in_=mt_v[:, 14:16, :])

            # the replicated-ones block of W: DVE memset, no deps,
            # scheduled right after the preamble.
            nc.vector.memset(w_sb[:, WONE:WONE + B, :], 1.0)

            # ---- stationary operand W = [hi | lo | sgn | ones], bf16, in 4
            # pipelined blocks of 4 chunks.  ln|x| = ln(x^2) (x^2 on DVE
            # avoids the Abs table); the 0.5 is folded into the Exp scale.
            # ACT does only the Ln's; DVE everything else.
            sq_sb = pool.tile([128, B, KC], F32)
            ln_sb = pool.tile([128, B, KC], F32)
            for h in range(0, KC, PB):
                sl = slice(h, h + PB)
                nc.vector.tensor_tensor(
                    out=sq_sb[:, :, sl], in0=x_sb[:, :, sl], in1=x_sb[:, :, sl],
                    op=AluOpType.mult)
                nc.scalar.activation(out=ln_sb[:, :, sl], in_=sq_sb[:, :, sl], func=AF.Ln)
                nc.vector.tensor_scalar(
                    out=w_sb[:, WHI:WHI + B, sl], in0=ln_sb[:, :, sl],
                    scalar1=0.0, scalar2=None, op0=AluOpType.add)
                nc.vector.tensor_tensor(
                    out=w_sb[:, WLO:WLO + B, sl], in0=ln_sb[:, :, sl],
                    in1=w_sb[:, WHI:WHI + B, sl], op=AluOpType.subtract)

            # dummy Exp AFTER the Ln phase (input reads ln_sb to pin the
            # ordering): its table load overlaps the matmuls instead of
            # stalling the real Exps.
            dex = pool.tile([1, 1], F32)
            nc.scalar.activation(out=dex, in_=ln_sb[0:1, 0:1, KC - 1], func=AF.Exp)

            # ---- main accumulation: ps[0:128] += W_c^T @ M_c over 16 chunks.
            # Rows 0:32 = Lhi, 32:64 = Llo, 64:96 = C, 96:128 = deg (x32).
            ps = psum_pool.tile([128, SHARD], F32)
            for c in range(KC):
                nc.tensor.matmul(
                    ps, lhsT=w_sb[:, :, c], rhs=m_sb[:, c, :],
                    start=(c == 0), stop=(c == KC - 1))

            # ---- epilogue.  PSUM readers of one bank serialize, so keep
            # them few and short, and put the Lhi+Llo sum BEFORE a single Exp:
            #   ci2  = 2*C as int32                     (PSUM reader 1)
            #   lo   = Llo drained to SBUF              (PSUM reader 2)
            #   ll   = Lhi + lo (one PSUM + one SBUF operand at equal base -
            #          mixed-space inputs skip the equal-base rule) (reader 3)
            #   zv   = min(deg,1) - (ci2 & 2)           (PSUM reader 4)
            #   out  = exp(.5*ll) * zv                  (ONE Exp, not two)
            # deg==0 implies C==0, so empty rows get exactly 0.
            lo_sb = pool.tile([B, SHARD], F32)
            nc.vector.tensor_scalar(
                out=lo_sb, in0=ps[WLO:WLO + B, :],
                scalar1=0.0, scalar2=None, op0=AluOpType.add)
            ll = pool.tile([B, SHARD], F32)
            nc.vector.tensor_tensor(
                out=ll, in0=ps[WHI:WHI + B, :], in1=lo_sb, op=AluOpType.add)
            ci2 = pool.tile([B, SHARD], I32)
            nc.vector.tensor_scalar(
                out=ci2, in0=ps[WSGN:WSGN + B, :],
                scalar1=2.0, scalar2=None, op0=AluOpType.mult)
            odd2 = pool.tile([B, SHARD], I32)
            nc.vector.tensor_scalar(
                out=odd2, in0=ci2, scalar1=2, scalar2=None,
                op0=AluOpType.bitwise_and)
            a = pool.tile([B, SHARD], F32)
            nc.scalar.activation(out=a, in_=ll, func=AF.Exp, scale=0.5)
            zv = pool.tile([B, SHARD], F32)
            nc.vector.scalar_tensor_tensor(
                out=zv, in0=ps[WONE:WONE + B, :], scalar=1.0, in1=odd2,
                op0=AluOpType.min, op1=AluOpType.subtract)
            o_sb = pool.tile([B, SHARD], F32)
            nc.vector.tensor_tensor(out=o_sb, in0=a, in1=zv, op=AluOpType.mult)
            # fp32 output (fp16 underflows: products reach ~1e-9, below the
            # fp16 subnormal floor); two halves on the two HWDGE rings
            H = SHARD // 2
            nc.sync.dma_start(out=out.ap()[:, 0:H], in_=o_sb[:, 0:H])
            nc.scalar.dma_start(out=out.ap()[:, H:SHARD], in_=o_sb[:, H:SHARD])

    nc.compile()
    return nc


def _get_program():
    global _PROG
    if _PROG is None:
        _PROG = _build_program()
    return _PROG


def _prep_inputs(x, mask):
    import ml_dtypes

    x = np.ascontiguousarray(x, dtype=np.float32)
    mask = np.ascontiguousarray(mask, dtype=np.float32)
    # xt[p, b*KC + c] = x[b, c*128 + p]
    xtf = np.ascontiguousarray(
        x.T.reshape(KC, 128, B).transpose(1, 2, 0).reshape(128, B * KC))
    st = (xtf < 0).astype(ml_dtypes.bfloat16)
    xt = xtf.astype(np.float16)
    mask_bf = mask.astype(ml_dtypes.bfloat16)
    in_maps = []
    for k in range(NCORES):
        shard = mask_bf[k * SHARD:(k + 1) * SHARD, :]      # [256, 2048]
        # mt[p, c*SHARD + n] = mask[k*SHARD + n, c*128 + p]
        mt = np.ascontiguousarray(
            shard.T.reshape(KC, 128, SHARD).transpose(1, 0, 2).reshape(128, KC * SHARD))
        in_maps.append({"xt": xt, "st": st, "mt": mt})
    return in_maps


def run(x, mask, trace=False):
    """Run on 8 NeuronCores; returns (output, BassKernelResults)."""
    from concourse.bass_utils import run_bass_kernel_spmd

    nc = _get_program()
    in_maps = _prep_inputs(x, mask)
    res = run_bass_kernel_spmd(nc, in_maps, core_ids=list(range(NCORES)), trace=trace)
    out = np.concatenate([np.asarray(r["out"], dtype=np.float32)
                          for r in res.results], axis=1)
    return np.ascontiguousarray(out, dtype=np.float32), res


def kernel(x, mask):
    out, _ = run(x, mask, trace=False)
    return out


# revision 41
# speedup vs baseline: 1.1827x; 1.0063x over previous
"""Trainium2 Bass kernel for nn_CnUpdateLayer (LDPC check-node update).

Math: out[b,i] = prod_{j: mask[i,j]!=0} x[b,j], or 0 if mask row i is empty.
Since mask is exactly {0,1} and x ~ randn (no exact zeros), the masked product
is computed in log-domain via matmul:

    L[b,i]  = sum_j ln|x[b,j]| * mask[i,j]       (magnitude, log domain)
    C[b,i]  = sum_j [x[b,j]<0] * mask[i,j]       (negative count)
    deg[i]  = sum_j mask[i,j]                    (row degree)
    out     = exp(.5*Lhi)*exp(.5*Llo) * (min(deg,1) - 2*(C&1))

ln(x^2) is split hi/lo into two bf16 halves so the matmul runs at bf16 rate
while keeping ~fp32 accuracy.  The stationary operand is
[ln_hi | ln_lo | signbits | ones*32] = 128 columns: the 32 identical "ones"
columns replicate the row degree into PSUM partitions 96:128, so the epilogue
needs no K=1 broadcast matmul.

Schedule (final):
  - The mask streams on the SWDGE (gpsimd) ring in consumption order as
    seven 2-chunk groups (first group 2 chunks as well); the final 2-chunk
    group rides the sync HWDGE ring, arriving long before it is needed.
    Matmuls are paced by per-transfer completion receipts (~1.3-2us after
    last byte): 128KB groups fire receipts every ~0.6us of stream instead
    of every 1.2us, halving every receipt stall including the phase start,
    and the shorter gpsimd stream ends (and receipts) sooner.
    Under load the SWDGE ring sustains 160-230 GB/s while each HWDGE ring
    manages only ~40-80.  x (fp16) + sign bits also ride the sync ring;
    the scalar ring stays free for ACT work until the output.  W is
    stored column-major [128, 128cols, KC] so the sign-bit DMA and ones
    memset write contiguous bytes per partition (a strided 64B-element dest
    shatters the DMA into tiny packets and chokes every ring) while each
    chunk's lhsT stays a single-stride AP.  Each transfer's semaphore fires
    ~1.3-2us after its last byte (completion receipt) - the matmul phase is
    paced by group receipts.
  - No PE warm-up: the hot 2.4GHz p-state is unreachable (HW DVFS needs
    sustained real utilization, so matmuls run at 1.2GHz), and a dummy
    bridge FIFO-delays the real matmuls for more than the ~180ns one
    cold-state matmul costs.
  - ln-prep: 4 pipelined blocks; DVE does squares + hi-casts + lo, ACT only
    the 4 Ln's.  GpSimd ALU is never used (~30x slower than DVE and its
    SBUF traffic interferes with DVE).
  - Epilogue (~2.4us, DVE-queue-bound): four PSUM readers in order
    lo, Lhi+lo, 2C->int32, min(deg,1)-(2C&2), then ONE exp(.5*(Lhi+Llo)).
    Mixed PSUM+SBUF operands at equal partition base are legal (the
    equal-base rule only binds when BOTH inputs are SBUF).  Output leaves
    in two halves on the two HWDGE rings.
"""

import sys

if "/opt/trn_rl_repo" not in sys.path:
    sys.path.insert(0, "/opt/trn_rl_repo")

import numpy as np

B = 32          # batch codewords
IN_F = 2048     # input edges
OUT_F = 2048    # output edges
NCORES = 8
SHARD = OUT_F // NCORES     # 256 output edges per core
KC = IN_F // 128            # 16 contraction chunks of 128
PB = 4                      # prep block size (chunks)
WHI, WLO, WSGN, WONE = 0, B, 2 * B, 3 * B       # 0, 32, 64, 96
WTOT = 4 * B                                    # 128

_PROG = None


def _build_program():
    import concourse.tile as tile
    from concourse import bacc, mybir
    from concourse.alu_op_type import AluOpType

    F32 = mybir.dt.float32
    F16 = mybir.dt.float16
    I32 = mybir.dt.int32
    BF16 = mybir.dt.bfloat16
    AF = mybir.ActivationFunctionType

    nc = bacc.Bacc("TRN2", target_bir_lowering=False)
    xt = nc.dram_tensor("xt", [128, KC * B], F16, kind="ExternalInput")
    st = nc.dram_tensor("st", [128, KC * B], BF16, kind="ExternalInput")
    mt = nc.dram_tensor("mt", [128, KC * SHARD], BF16, kind="ExternalInput")
    out = nc.dram_tensor("out", [B, SHARD], F32, kind="ExternalOutput")

    with tile.TileContext(nc) as tc:
        with (
            tc.tile_pool(name="pool", bufs=1) as pool,
            tc.tile_pool(name="psum", bufs=1, space="PSUM") as psum_pool,
        ):
            # ---- input DMAs.  x + sign bits on the scalar HWDGE ring (ahead
            # of the Ln table load, which overlaps their transfer); the mask
            # alone on the sync ring in 4 groups of 4 chunks so arrival order
            # matches matmul consumption order.
            # dummy Ln first on the scalar queue: its 1.28us ACT_TABLE_LOAD
            # runs while x streams on the sync ring, and naturally delays the
            # scalar-ring mask issues so x gets the full DMA bandwidth first.
            dmy = pool.tile([1, 1], F32)
            nc.vector.memset(dmy, 1.0)
            dln = pool.tile([1, 1], F32)
            nc.scalar.activation(out=dln, in_=dmy, func=AF.Ln)

            x_sb = pool.tile([128, B, KC], F16)
            nc.sync.dma_start(
                out=x_sb, in_=xt.ap().rearrange("p (b c) -> p b c", c=KC))
            w_sb = pool.tile([128, WTOT, KC], BF16)
            nc.sync.dma_start(
                out=w_sb[:, WSGN:WSGN + B, :],
                in_=st.ap().rearrange("p (b c) -> p b c", c=KC))
            m_sb = pool.tile([128, KC, SHARD], BF16)
            mt_v = mt.ap().rearrange("p (c n) -> p c n", n=SHARD)
            nc.gpsimd.dma_start(out=m_sb[:, 0:2, :], in_=mt_v[:, 0:2, :])
            nc.gpsimd.dma_start(out=m_sb[:, 2:4, :], in_=mt_v[:, 2:4, :])
            nc.gpsimd.dma_start(out=m_sb[:, 4:6, :], in_=mt_v[:, 4:6, :])
            nc.gpsimd.dma_start(out=m_sb[:, 6:8, :], in_=mt_v[:, 6:8, :])
            nc.gpsimd.dma_start(out=m_sb[:, 8:10, :], in_=mt_v[:, 8:10, :])
            nc.gpsimd.dma_start(out=m_sb[:, 10:12, :], in_=mt_v[:, 10:12, :])
            nc.scalar.dma_start(out=m_sb[:, 12:14, :], in_=mt_v[:, 12:14, :])
            nc.sync.dma_start(out=m_sb[:, 14:16, :], in_=mt_v[:, 14:16, :])

            # the replicated-ones block of W: DVE memset, no deps,
            # scheduled right after the preamble.
            nc.vector.memset(w_sb[:, WONE:WONE + B, :], 1.0)

            # ---- stationary operand W = [hi | lo | sgn | ones], bf16, in 4
            # pipelined blocks of 4 chunks.  ln|x| = ln(x^2) (x^2 on DVE
            # avoids the Abs table); the 0.5 is folded into the Exp scale.
            # ACT does only the Ln's; DVE everything else.
            sq_sb = pool.tile([128, B, KC], F32)
            ln_sb = pool.tile([128, B, KC], F32)
            for h in range(0, KC, PB):
                sl = slice(h, h + PB)
                nc.vector.tensor_tensor(
                    out=sq_sb[:, :, sl], in0=x_sb[:, :, sl], in1=x_sb[:, :, sl],
                    op=AluOpType.mult)
                nc.scalar.activation(out=ln_sb[:, :, sl], in_=sq_sb[:, :, sl], func=AF.Ln)
                nc.vector.tensor_scalar(
                    out=w_sb[:, WHI:WHI + B, sl], in0=ln_sb[:, :, sl],
                    scalar1=0.0, scalar2=None, op0=AluOpType.add)
                nc.vector.tensor_tensor(
                    out=w_sb[:, WLO:WLO + B, sl], in0=ln_sb[:, :, sl],
                    in1=w_sb[:, WHI:WHI + B, sl], op=AluOpType.subtract)

            # dummy Exp AFTER the Ln phase (input reads ln_sb to pin the
            # ordering): its table load overlaps the matmuls instead of
            # stalling the real Exps.
            dex = pool.tile([1, 1], F32)
            nc.scalar.activation(out=dex, in_=ln_sb[0:1, 0:1, KC - 1], func=AF.Exp)

            # ---- main accumulation: ps[0:128] += W_c^T @ M_c over 16 chunks.
            # Rows 0:32 = Lhi, 32:64 = Llo, 64:96 = C, 96:128 = deg (x32).
            ps = psum_pool.tile([128, SHARD], F32)
            for c in range(KC):
                nc.tensor.matmul(
                    ps, lhsT=w_sb[:, :, c], rhs=m_sb[:, c, :],
                    start=(c == 0), stop=(c == KC - 1))

            # ---- epilogue.  PSUM readers of one bank serialize, so keep
            # them few and short, and put the Lhi+Llo sum BEFORE a single Exp:
            #   ci2  = 2*C as int32                     (PSUM reader 1)
            #   lo   = Llo drained to SBUF              (PSUM reader 2)
            #   ll   = Lhi + lo (one PSUM + one SBUF operand at equal base -
            #          mixed-space inputs skip the equal-base rule) (reader 3)
            #   zv   = min(deg,1) - (ci2 & 2)           (PSUM reader 4)
            #   out  = exp(.5*ll) * zv                  (ONE Exp, not two)
            # deg==0 implies C==0, so empty rows get exactly 0.
            lo_sb = pool.tile([B, SHARD], F32)
            nc.vector.tensor_scalar(
                out=lo_sb, in0=ps[WLO:WLO + B, :],
                scalar1=0.0, scalar2=None, op0=AluOpType.add)
            ll = pool.tile([B, SHARD], F32)
            nc.vector.tensor_tensor(
                out=ll, in0=ps[WHI:WHI + B, :], in1=lo_sb, op=AluOpType.add)
            ci2 = pool.tile([B, SHARD], I32)
            nc.vector.tensor_scalar(
                out=ci2, in0=ps[WSGN:WSGN + B, :],
                scalar1=2.0, scalar2=None, op0=AluOpType.mult)
            odd2 = pool.tile([B, SHARD], I32)
            nc.vector.tensor_scalar(
                out=odd2, in0=ci2, scalar1=2, scalar2=None,
                op0=AluOpType.bitwise_and)
            a = pool.tile([B, SHARD], F32)
            nc.scalar.activation(out=a, in_=ll, func=AF.Exp, scale=0.5)
            zv = pool.tile([B, SHARD], F32)
            nc.vector.scalar_tensor_tensor(
                out=zv, in0=ps[WONE:WONE + B, :], scalar=1.0, in1=odd2,
                op0=AluOpType.min, op1=AluOpType.subtract)
            o_sb = pool.tile([B, SHARD], F32)
            nc.vector.tensor_tensor(out=o_sb, in0=a, in1=zv, op=AluOpType.mult)
            # fp32 output (fp16 underflows: products reach ~1e-9, below the
            # fp16 subnormal floor); two halves on the two HWDGE rings
            H = SHARD // 2
            nc.sync.dma_start(out=out.ap()[:, 0:H], in_=o_sb[:, 0:H])
            nc.scalar.dma_start(out=out.ap()[:, H:SHARD], in_=o_sb[:, H:SHARD])

    nc.compile()
    return nc


def _get_program():
    global _PROG
    if _PROG is None:
        _PROG = _build_program()
    return _PROG


def _prep_inputs(x, mask):
    import ml_dtypes

    x = np.ascontiguousarray(x, dtype=np.float32)
    mask = np.ascontiguousarray(mask, dtype=np.float32)
    # xt[p, b*KC + c] = x[b, c*128 + p]
    xtf = np.ascontiguousarray(
        x.T.reshape(KC, 128, B).transpose(1, 2, 0).reshape(128, B * KC))
    st = (xtf < 0).astype(ml_dtypes.bfloat16)
    xt = xtf.astype(np.float16)
    mask_bf = mask.astype(ml_dtypes.bfloat16)
    in_maps = []
    for k in range(NCORES):
        shard = mask_bf[k * SHARD:(k + 1) * SHARD, :]      # [256, 2048]
        # mt[p, c*SHARD + n] = mask[k*SHARD + n, c*128 + p]
        mt = np.ascontiguousarray(
            shard.T.reshape(KC, 128, SHARD).transpose(1, 0, 2).reshape(128, KC * SHARD))
        in_maps.append({"xt": xt, "st": st, "mt": mt})
    return in_maps


def run(x, mask, trace=False):
    """Run on 8 NeuronCores; returns (output, BassKernelResults)."""
    from concourse.bass_utils import run_bass_kernel_spmd

    nc = _get_program()
    in_maps = _prep_inputs(x, mask)
    res = run_bass_kernel_spmd(nc, in_maps, core_ids=list(range(NCORES)), trace=trace)
    out = np.concatenate([np.asarray(r["out"], dtype=np.float32)
                          for r in res.results], axis=1)
    return np.ascontiguousarray(out, dtype=np.float32), res


def kernel(x, mask):
    out, _ = run(x, mask, trace=False)
    return out
